# revision 1
# baseline (speedup 1.0000x reference)
"""Trainium2 Bass kernel for MixedIntQuantizedLinear.

Computation (see reference):
  W_dq[o,i] = W_int[o,i] * (scale_i32[o, i//64] / 2^24)
  per-token: amax_t = clip(max|x_t|, 1e-8); s_t = amax_t/127
             q_t = round(x_t / s_t)  (|q| <= 127, round-to-nearest-even)
  y[t,o] = s_t * sum_i q_t[i] * W_dq[o,i] + bias[o]

Sharding over 8 NeuronCores: 2 token-groups (batch halves) x 4
out-feature groups of 1024.  Each core computes y_core [4096, 1024].

Per-core kernel strategy (v4):
  - W ships as int8 (lossless: values in [-127,127]) to quarter DMA
    bytes.  Device: ScalarE int8->fp32, DVE multiply by broadcast block
    scales -> bf16, XBAR DMA-transpose into resident WT tensors.
    W is processed in quarter-stripes, k-major, into 8 separate WT
    tensors [128, 8, 512] so the first matmuls start after ~1/32 of
    W-prep instead of all of it.  PE does matmuls only.
  - x tiles [128 tok, 4096]: DVE absmax-reduce; quantize via the fp32
    magic-number trick (x*inv + 1.5*2^23 rounds to int with plain fp32
    RNE); ScalarE subtracts the magic and emits exact-integer bf16;
    one XBAR DMA-transpose produces qT [128, 32, 128].
  - 32 accumulating bf16 matmuls per PSUM tile [128 tok, 512 out];
    epilogue: ScalarE copy with per-partition scale s_t, DVE adds the
    (PE-broadcast) bias row, DMA out.
"""

import os
import sys

sys.path.insert(0, "/opt/trn_rl_repo")

import numpy as np

import concourse.bass as bass
import concourse.tile as tile
from concourse import bacc, mybir
from concourse.bass_utils import run_bass_kernel_spmd

P = 128
IN_F = 4096
OUT_F = 4096
TOKENS = 8192          # 4 * 2048
N_CORES = 8
TG = 2                 # token groups
OG = 4                 # out-feature groups
T_CORE = TOKENS // TG  # 4096 tokens per core
O_CORE = OUT_F // OG   # 1024 out features per core
KT = IN_F // P         # 32 contraction tiles
TT = T_CORE // P       # 32 token tiles
OC = O_CORE // 512     # 2 psum chunks of 512
BLOCK = 64
MAGIC = 12582912.0     # 1.5 * 2^23: fp32 round-to-int magic constant
INV_SCALE_SHIFT = 1.0 / (1 << 24)

QF = IN_F // 4         # 1024: quarter-stripe width
QB = QF // BLOCK       # 16 blocks per quarter
QK = KT // 4           # 8 k-tiles per quarter

F32 = mybir.dt.float32
BF16 = mybir.dt.bfloat16
I32 = mybir.dt.int32
I8 = mybir.dt.int8
ACT_COPY = mybir.ActivationFunctionType.Copy


def build_kernel(reps=1, dyn_loop_n=None, ablate=None):
    """dyn_loop_n: if set, wrap the token loop in a hardware For_i loop
    with that trip count (benchmarking only)."""
    nc = bacc.Bacc(None, target_bir_lowering=False, debug=False)

    x_d = nc.dram_tensor("x", [T_CORE, IN_F], F32, kind="ExternalInput")
    w_d = nc.dram_tensor("w", [O_CORE, IN_F], I8, kind="ExternalInput")
    s_d = nc.dram_tensor("s", [O_CORE, BLOCK], I32, kind="ExternalInput")
    b_d = nc.dram_tensor("b", [1, O_CORE], F32, kind="ExternalInput")
    y_d = nc.dram_tensor("y", [T_CORE, O_CORE], F32, kind="ExternalOutput")

    with tile.TileContext(nc) as tc:
        with (
            tc.tile_pool(name="const", bufs=1) as const_pool,
            tc.tile_pool(name="wt", bufs=1) as wt_pool,
            tc.tile_pool(name="psum_y", bufs=int(os.environ.get("KERNEL_PSUM", "6")), space="PSUM") as psum_y,
            tc.tile_pool(name="psum_misc", bufs=max(1, int(os.environ.get("KERNEL_PSUMM", "2"))), space="PSUM") as psum_misc_real,
        ):
            psum_misc = psum_y if os.environ.get("KERNEL_PSUMM") == "0" \
                else psum_misc_real
            # ---- bias broadcast row -> [128, O_CORE] via K=1 matmul ----
            ones_k1 = const_pool.tile([1, P], F32)
            nc.vector.memset(ones_k1[:], 1.0)
            bias_sb = const_pool.tile([1, O_CORE], F32)
            nc.sync.dma_start(bias_sb[:], b_d[:])
            bias_bcast = const_pool.tile([P, O_CORE], F32)
            for oc in range(OC):
                pb = psum_misc.tile([P, 512], F32, tag="py" if psum_misc is psum_y else "ptr")
                nc.tensor.matmul(
                    pb[:], ones_k1[:], bias_sb[:, oc * 512:(oc + 1) * 512],
                    start=True, stop=True,
                )
                nc.scalar.copy(bias_bcast[:, oc * 512:(oc + 1) * 512], pb[:])

            # ---- block scales -> fp32 [128, 8, 64] ----
            n_str = O_CORE // P  # 8 weight stripes
            sc_i32 = const_pool.tile([P, n_str, BLOCK], I32)
            nc.sync.dma_start(
                sc_i32[:], s_d.rearrange("(s p) b -> p s b", p=P)
            )
            sc_f32 = const_pool.tile([P, n_str, BLOCK], F32)
            nc.vector.tensor_copy(sc_f32[:], sc_i32[:])
            nc.vector.tensor_scalar_mul(sc_f32[:], sc_f32[:], INV_SCALE_SHIFT)

            # WT tensors, one per (oc chunk, k quarter): [128, QK, 512]
            wtq = [[None] * 4 for _ in range(OC)]
            for oc in range(OC):
                for kq in range(4):
                    w = wt_pool.tile([P, QK, 512], BF16, name=f"wt{oc}_{kq}",
                                     tag=f"wt{oc}_{kq}")
                    wtq[oc][kq] = w

            with (
                tc.tile_pool(name="wprep", bufs=2) as wprep,
                tc.tile_pool(name="xin", bufs=int(os.environ.get("KERNEL_XIN", "3"))) as xin_pool,
                tc.tile_pool(name="small", bufs=6) as small_pool,
                tc.tile_pool(name="qb", bufs=int(os.environ.get("KERNEL_QB", "3"))) as qb_pool,
                tc.tile_pool(name="qt", bufs=3) as qt_pool,
                tc.tile_pool(name="orow", bufs=3) as orow_pool,
            ):
                # prefetch first x tiles so quant pipeline overlaps W-prep
                PREFETCH = 0 if dyn_loop_n is not None else 2
                xts = {}
                for tt in range(PREFETCH):
                    xt = xin_pool.tile([P, IN_F], F32, tag="xt")
                    nc.sync.dma_start(xt[:], x_d[tt * P:(tt + 1) * P, :])
                    xts[tt] = xt

                # ---- W: int8 -> fp32 -> dequant bf16 -> XBAR into WT ----
                # k-quarter-major so wtq[0][0] completes first.
                for kq in range(4):
                    for s in range(n_str):
                        w_i8 = wprep.tile([P, QF], I8, tag="w_i8")
                        nc.sync.dma_start(
                            w_i8[:],
                            w_d[s * P:(s + 1) * P, kq * QF:(kq + 1) * QF])
                        w_f32 = wprep.tile([P, QF], F32, tag="w_f32")
                        nc.scalar.copy(w_f32[:], w_i8[:])
                        w_bf = wprep.tile([P, QF], BF16, tag="w_bf")
                        nc.vector.tensor_tensor(
                            w_bf.rearrange("p (nb j) -> p nb j", j=BLOCK),
                            w_f32.rearrange("p (nb j) -> p nb j", j=BLOCK),
                            sc_f32[:, s, kq * QB:(kq + 1) * QB, None]
                            .to_broadcast((P, QB, BLOCK)),
                            mybir.AluOpType.mult,
                        )
                        nc.sync.dma_start_transpose(
                            wtq[s // 4][kq][:, :, (s % 4) * P:(s % 4 + 1) * P],
                            w_bf[:],
                        )

                # ---- main token loop ----
                static_qt = []; static_st = []
                PAIR = int(os.environ.get("KERNEL_PAIR", "0"))
                xq = {"sync": nc.sync, "gpsimd": nc.gpsimd,
                      "scalar": nc.scalar}[os.environ.get("KERNEL_XQ", "sync")]
                oq = {"sync": nc.sync, "gpsimd": nc.gpsimd,
                      "scalar": nc.scalar}[os.environ.get("KERNEL_OQ", "sync")]
                tq = {"sync": nc.sync,
                      "scalar": nc.scalar}[os.environ.get("KERNEL_TQ", "sync")]
                QB_DVE = int(os.environ.get("KERNEL_QB_DVE", "0"))
                def token_loop(first_pass):
                    if not PAIR:
                        for tt in range(TT):
                            _token_tile(tt, first_pass)
                        return
                    for tp in range(0, TT, 2):
                        pair = (tp, tp + 1)
                        xts_p, sts, qts = {}, {}, {}
                        for tt in pair:
                            if tt in xts and first_pass:
                                xt = xts.pop(tt)
                            else:
                                xt = xin_pool.tile([P, IN_F], F32, tag="xt")
                                nc.sync.dma_start(
                                    xt[:], x_d[tt * P:(tt + 1) * P, :])
                            xts_p[tt] = xt
                        for tt in pair:
                            xt = xts_p[tt]
                            amax = small_pool.tile([P, 1], F32, tag="amax")
                            nc.vector.tensor_reduce(
                                amax[:], xt[:], axis=mybir.AxisListType.X,
                                op=mybir.AluOpType.max,
                                apply_absolute_value=True,
                            )
                            s_t = small_pool.tile([P, 1], F32, tag="s_t")
                            nc.vector.tensor_scalar(
                                s_t[:], amax[:], 1e-8, 1.0 / 127.0,
                                op0=mybir.AluOpType.max,
                                op1=mybir.AluOpType.mult,
                            )
                            inv = small_pool.tile([P, 1], F32, tag="inv")
                            nc.vector.reciprocal(inv[:], s_t[:])
                            sts[tt] = s_t
                            nc.vector.tensor_scalar(
                                xt[:], xt[:], inv[:], MAGIC,
                                op0=mybir.AluOpType.mult,
                                op1=mybir.AluOpType.add,
                            )
                            qb = qb_pool.tile([P, IN_F], BF16, tag="qb")
                            nc.scalar.activation(qb[:], xt[:], ACT_COPY,
                                                 bias=-MAGIC)
                            qts[tt] = qb
                        for tt in pair:
                            qt = qt_pool.tile([P, KT, P], BF16, tag="qt")
                            nc.sync.dma_start_transpose(qt[:], qts[tt][:])
                            qts[tt] = qt
                        for tt in pair:
                            qt, s_t = qts[tt], sts[tt]
                            orow = orow_pool.tile([P, O_CORE], F32, tag="orow")
                            for oc in range(OC):
                                py = psum_y.tile([P, 512], F32, tag="py")
                                for k in range(KT):
                                    nc.tensor.matmul(
                                        py[:], qt[:, k, :],
                                        wtq[oc][k // QK][:, k % QK, :],
                                        start=(k == 0), stop=(k == KT - 1),
                                    )
                                nc.scalar.activation(
                                    orow[:, oc * 512:(oc + 1) * 512], py[:],
                                    ACT_COPY, scale=s_t[:],
                                )
                                nc.vector.tensor_tensor(
                                    orow[:, oc * 512:(oc + 1) * 512],
                                    orow[:, oc * 512:(oc + 1) * 512],
                                    bias_bcast[:, oc * 512:(oc + 1) * 512],
                                    mybir.AluOpType.add,
                                )
                            nc.sync.dma_start(
                                y_d[tt * P:(tt + 1) * P, :], orow[:])

                def _token_tile(tt, first_pass):
                    mm_only = ablate == 'mmonly'
                    no_mm = ablate == 'nomm'
                    if mm_only and static_qt:
                        qt = static_qt[0]; s_t = static_st[0]
                        orow = orow_pool.tile([P, O_CORE], F32, tag="orow")
                        for oc in range(OC):
                            py = psum_y.tile([P, 512], F32, tag="py")
                            for k in range(KT):
                                nc.tensor.matmul(
                                    py[:], qt[:, k, :],
                                    wtq[oc][k // QK][:, k % QK, :],
                                    start=(k == 0), stop=(k == KT - 1),
                                )
                            nc.scalar.activation(
                                orow[:, oc * 512:(oc + 1) * 512], py[:],
                                ACT_COPY, scale=s_t[:],
                            )
                            nc.vector.tensor_tensor(
                                orow[:, oc * 512:(oc + 1) * 512],
                                orow[:, oc * 512:(oc + 1) * 512],
                                bias_bcast[:, oc * 512:(oc + 1) * 512],
                                mybir.AluOpType.add,
                            )
                        nc.sync.dma_start(y_d[tt * P:(tt + 1) * P, :], orow[:])
                        return
                    if tt in xts and first_pass:
                        xt = xts.pop(tt)
                    else:
                        xt = xin_pool.tile([P, IN_F], F32, tag="xt")
                        xq.dma_start(xt[:], x_d[tt * P:(tt + 1) * P, :])

                    amax = small_pool.tile([P, 1], F32, tag="amax")
                    nc.vector.tensor_reduce(
                        amax[:], xt[:], axis=mybir.AxisListType.X,
                        op=mybir.AluOpType.max, apply_absolute_value=True,
                    )
                    nc.vector.tensor_scalar_max(amax[:], amax[:], 1e-8)
                    s_t = small_pool.tile([P, 1], F32, tag="s_t")
                    nc.vector.tensor_scalar_mul(s_t[:], amax[:], 1.0 / 127.0)
                    inv = small_pool.tile([P, 1], F32, tag="inv")
                    nc.vector.reciprocal(inv[:], s_t[:])

                    # x <- x * inv + MAGIC  (fp32; integer part = q + MAGIC)
                    nc.vector.tensor_scalar(
                        xt[:], xt[:], inv[:], MAGIC,
                        op0=mybir.AluOpType.mult, op1=mybir.AluOpType.add,
                    )
                    # q (exact small ints) in bf16
                    qb = qb_pool.tile([P, IN_F], BF16, tag="qb")
                    if QB_DVE:
                        nc.vector.tensor_scalar_add(qb[:], xt[:], -MAGIC)
                    else:
                        nc.scalar.activation(qb[:], xt[:], ACT_COPY,
                                             bias=-MAGIC)

                    # XBAR transpose -> qT [128(i), KT, 128(t)]
                    qt = qt_pool.tile([P, KT, P], BF16, tag="qt")
                    if int(os.environ.get("KERNEL_TSPLIT", "0")):
                        h = IN_F // 2
                        tq.dma_start_transpose(qt[:, :KT // 2, :], qb[:, :h])
                        tq.dma_start_transpose(qt[:, KT // 2:, :], qb[:, h:])
                    else:
                        tq.dma_start_transpose(qt[:], qb[:])

                    if mm_only and not static_qt:
                        static_qt.append(qt); static_st.append(s_t)
                    if no_mm:
                        return
                    orow = orow_pool.tile([P, O_CORE], F32, tag="orow")
                    for oc in range(OC):
                        py = psum_y.tile([P, 512], F32, tag="py")
                        for k in range(KT):
                            nc.tensor.matmul(
                                py[:], qt[:, k, :],
                                wtq[oc][k // QK][:, k % QK, :],
                                start=(k == 0), stop=(k == KT - 1),
                            )
                        nc.scalar.activation(
                            orow[:, oc * 512:(oc + 1) * 512], py[:],
                            ACT_COPY, scale=s_t[:],
                        )
                        nc.vector.tensor_tensor(
                            orow[:, oc * 512:(oc + 1) * 512],
                            orow[:, oc * 512:(oc + 1) * 512],
                            bias_bcast[:, oc * 512:(oc + 1) * 512],
                            mybir.AluOpType.add,
                        )
                    oq.dma_start(y_d[tt * P:(tt + 1) * P, :], orow[:])

                if dyn_loop_n is not None:
                    if ablate == 'mmonly':
                        _token_tile(0, False)   # fills static_qt
                    with tc.For_i(0, dyn_loop_n, 1):
                        token_loop(first_pass=False)
                else:
                    for rep in range(reps):
                        token_loop(first_pass=(rep == 0))

    nc.compile()
    return nc


_NC_CACHE = None


def _get_nc():
    global _NC_CACHE
    if _NC_CACHE is None:
        _NC_CACHE = build_kernel()
    return _NC_CACHE


def kernel(x, W_int, scale_i32, bias, _trace=False, _tmpdir=None):
    nc = _get_nc()
    x2 = np.ascontiguousarray(x, dtype=np.float32).reshape(TOKENS, IN_F)
    W_i8 = np.asarray(W_int).astype(np.int8)          # lossless: [-127,127]
    scale_i32 = np.asarray(scale_i32, dtype=np.int32)
    bias2 = np.asarray(bias, dtype=np.float32).reshape(1, OUT_F)

    in_maps = []
    for c in range(N_CORES):
        tg, og = c // OG, c % OG
        in_maps.append({
            "x": np.ascontiguousarray(x2[tg * T_CORE:(tg + 1) * T_CORE]),
            "w": np.ascontiguousarray(W_i8[og * O_CORE:(og + 1) * O_CORE]),
            "s": np.ascontiguousarray(
                scale_i32[og * O_CORE:(og + 1) * O_CORE]),
            "b": np.ascontiguousarray(bias2[:, og * O_CORE:(og + 1) * O_CORE]),
        })

    res = run_bass_kernel_spmd(
        nc, in_maps, core_ids=list(range(N_CORES)),
        trace=_trace, tmpdir=_tmpdir,
    )
    y = np.empty((TOKENS, OUT_F), dtype=np.float32)
    for c in range(N_CORES):
        tg, og = c // OG, c % OG
        y[tg * T_CORE:(tg + 1) * T_CORE, og * O_CORE:(og + 1) * O_CORE] = \
            res.results[c]["y"]
    out = y.reshape(4, 2048, OUT_F)
    if _trace:
        return out, res
    return out



# revision 6
# speedup vs baseline: 1.0196x; 1.0196x over previous
"""Trainium2 Bass kernel for MixedIntQuantizedLinear.

Computation (see reference):
  W_dq[o,i] = W_int[o,i] * (scale_i32[o, i//64] / 2^24)
  per-token: amax_t = clip(max|x_t|, 1e-8); s_t = amax_t/127
             q_t = round(x_t / s_t)  (|q| <= 127, round-to-nearest-even)
  y[t,o] = s_t * sum_i q_t[i] * W_dq[o,i] + bias[o]

Sharding over 8 NeuronCores: 2 token-groups (batch halves) x 4
out-feature groups of 1024.  Each core computes y_core [4096, 1024].

Per-core kernel strategy (v5):
  - W ships as int8 (lossless: values in [-127,127]).  Device W-prep is
    a 2-stage chain per [128, 1024] stripe: DVE dequant (int8 in,
    broadcast block scales, bf16 out) then XBAR DMA-transpose into the
    resident k-major WT tensors.  W DMAs ride the gpsimd (SWDGE) queue,
    W transposes the sync queue, so the chain never head-of-line blocks
    the token pipeline.  bufs=6 keeps ~6 stripes in flight so the chain
    runs at XBAR throughput (~1.3us/stripe) instead of latency-bound.
  - Emission interleaves W-prep stripe groups (4 stripes, kq-major)
    with the first 8 token tiles so every FIFO queue's program order
    matches data-readiness order.
  - x tiles [128 tok, 4096]: DVE absmax-reduce; quantize via the fp32
    magic-number trick (x*inv + 1.5*2^23 rounds to int with plain fp32
    RNE); ScalarE subtracts the magic and emits exact-integer bf16;
    one XBAR DMA-transpose (scalar queue) produces qT [128, 32, 128].
  - 32 accumulating bf16 matmuls per PSUM tile [128 tok, 512 out];
    epilogue: ScalarE copy with per-partition scale s_t, DVE adds the
    (PE-broadcast) bias row, DMA out on sync.
"""

import os
import sys

sys.path.insert(0, "/opt/trn_rl_repo")

import numpy as np

import concourse.bass as bass
import concourse.tile as tile
from concourse import bacc, mybir
from concourse.bass_utils import run_bass_kernel_spmd

P = 128
IN_F = 4096
OUT_F = 4096
TOKENS = 8192          # 4 * 2048
N_CORES = 8
TG = 2                 # token groups
OG = 4                 # out-feature groups
T_CORE = TOKENS // TG  # 4096 tokens per core
O_CORE = OUT_F // OG   # 1024 out features per core
KT = IN_F // P         # 32 contraction tiles
TT = T_CORE // P       # 32 token tiles
OC = O_CORE // 512     # 2 psum chunks of 512
BLOCK = 64
MAGIC = 12582912.0     # 1.5 * 2^23: fp32 round-to-int magic constant
INV_SCALE_SHIFT = 1.0 / (1 << 24)

QF = IN_F // 4         # 1024: quarter-stripe width
QB = QF // BLOCK       # 16 blocks per quarter
QK = KT // 4           # 8 k-tiles per quarter

F32 = mybir.dt.float32
BF16 = mybir.dt.bfloat16
I32 = mybir.dt.int32
I8 = mybir.dt.int8
ACT_COPY = mybir.ActivationFunctionType.Copy


def build_kernel():
    nc = bacc.Bacc(None, target_bir_lowering=False, debug=False)

    x_d = nc.dram_tensor("x", [T_CORE, IN_F], F32, kind="ExternalInput")
    w_d = nc.dram_tensor("w", [O_CORE, IN_F], I8, kind="ExternalInput")
    s_d = nc.dram_tensor("s", [O_CORE, BLOCK], I32, kind="ExternalInput")
    b_d = nc.dram_tensor("b", [1, O_CORE], F32, kind="ExternalInput")
    y_d = nc.dram_tensor("y", [T_CORE, O_CORE], F32, kind="ExternalOutput")

    WPREP_BUFS = int(os.environ.get("KERNEL_WPREP", "6"))
    XIN_BUFS = int(os.environ.get("KERNEL_XIN", "4"))
    QB_BUFS = int(os.environ.get("KERNEL_QB", "2"))
    QT_BUFS = int(os.environ.get("KERNEL_QT", "3"))
    OROW_BUFS = int(os.environ.get("KERNEL_OROW", "2"))
    PSUM_BUFS = int(os.environ.get("KERNEL_PSUM", "6"))

    with tile.TileContext(nc) as tc:
        with (
            tc.tile_pool(name="const", bufs=1) as const_pool,
            tc.tile_pool(name="wt", bufs=1) as wt_pool,
            tc.tile_pool(name="psum_y", bufs=PSUM_BUFS, space="PSUM") as psum_y,
            tc.tile_pool(name="psum_misc", bufs=2, space="PSUM") as psum_misc,
        ):
            # ---- bias broadcast row -> [128, O_CORE] via K=1 matmul ----
            ones_k1 = const_pool.tile([1, P], F32)
            nc.vector.memset(ones_k1[:], 1.0)
            bias_sb = const_pool.tile([1, O_CORE], F32)
            nc.sync.dma_start(bias_sb[:], b_d[:])
            bias_bcast = const_pool.tile([P, O_CORE], F32)
            for oc in range(OC):
                pb = psum_misc.tile([P, 512], F32, tag="pb")
                nc.tensor.matmul(
                    pb[:], ones_k1[:], bias_sb[:, oc * 512:(oc + 1) * 512],
                    start=True, stop=True,
                )
                nc.scalar.copy(bias_bcast[:, oc * 512:(oc + 1) * 512], pb[:])

            # ---- block scales -> fp32 [128, 8, 64] ----
            n_str = O_CORE // P  # 8 weight stripes per k-quarter
            sc_i32 = const_pool.tile([P, n_str, BLOCK], I32)
            nc.sync.dma_start(
                sc_i32[:], s_d.rearrange("(s p) b -> p s b", p=P)
            )
            sc_f32 = const_pool.tile([P, n_str, BLOCK], F32)
            nc.vector.tensor_copy(sc_f32[:], sc_i32[:])
            nc.vector.tensor_scalar_mul(sc_f32[:], sc_f32[:], INV_SCALE_SHIFT)

            # WT tensors, one per (oc chunk, k quarter): [128, QK, 512]
            wtq = [[None] * 4 for _ in range(OC)]
            for oc in range(OC):
                for kq in range(4):
                    w = wt_pool.tile([P, QK, 512], BF16, name=f"wt{oc}_{kq}",
                                     tag=f"wt{oc}_{kq}")
                    wtq[oc][kq] = w

            with (
                tc.tile_pool(name="wprep", bufs=WPREP_BUFS) as wprep,
                tc.tile_pool(name="xin", bufs=XIN_BUFS) as xin_pool,
                tc.tile_pool(name="small", bufs=6) as small_pool,
                tc.tile_pool(name="qb", bufs=QB_BUFS) as qb_pool,
                tc.tile_pool(name="qt", bufs=QT_BUFS) as qt_pool,
                tc.tile_pool(name="orow", bufs=OROW_BUFS) as orow_pool,
            ):
                xts = {}

                def emit_x_load(tt):
                    if tt >= TT or tt in xts:
                        return
                    xt = xin_pool.tile([P, IN_F], F32, tag="xt")
                    nc.sync.dma_start(xt[:], x_d[tt * P:(tt + 1) * P, :])
                    xts[tt] = xt

                # prefetch first x tiles so quant pipeline overlaps W-prep
                emit_x_load(0)
                emit_x_load(1)

                def emit_stripe(stripe):
                    # kq-major: stripe // n_str = kq, stripe % n_str = s
                    kq, s = stripe // n_str, stripe % n_str
                    w_i8 = wprep.tile([P, QF], I8, tag="w_i8")
                    nc.gpsimd.dma_start(
                        w_i8[:],
                        w_d[s * P:(s + 1) * P, kq * QF:(kq + 1) * QF])
                    w_bf = wprep.tile([P, QF], BF16, tag="w_bf")
                    nc.vector.tensor_tensor(
                        w_bf.rearrange("p (nb j) -> p nb j", j=BLOCK),
                        w_i8.rearrange("p (nb j) -> p nb j", j=BLOCK),
                        sc_f32[:, s, kq * QB:(kq + 1) * QB, None]
                        .to_broadcast((P, QB, BLOCK)),
                        mybir.AluOpType.mult,
                    )
                    nc.sync.dma_start_transpose(
                        wtq[s // 4][kq][:, :, (s % 4) * P:(s % 4 + 1) * P],
                        w_bf[:],
                    )

                def emit_token_tile(tt):
                    xt = xts.pop(tt)
                    amax = small_pool.tile([P, 1], F32, tag="amax")
                    nc.vector.tensor_reduce(
                        amax[:], xt[:], axis=mybir.AxisListType.X,
                        op=mybir.AluOpType.max, apply_absolute_value=True,
                    )
                    nc.vector.tensor_scalar_max(amax[:], amax[:], 1e-8)
                    s_t = small_pool.tile([P, 1], F32, tag="s_t")
                    nc.vector.tensor_scalar_mul(s_t[:], amax[:], 1.0 / 127.0)
                    inv = small_pool.tile([P, 1], F32, tag="inv")
                    nc.vector.reciprocal(inv[:], s_t[:])

                    # x <- x * inv + MAGIC  (fp32; integer part = q + MAGIC)
                    nc.vector.tensor_scalar(
                        xt[:], xt[:], inv[:], MAGIC,
                        op0=mybir.AluOpType.mult, op1=mybir.AluOpType.add,
                    )
                    # q (exact small ints) in bf16
                    qb = qb_pool.tile([P, IN_F], BF16, tag="qb")
                    nc.scalar.activation(qb[:], xt[:], ACT_COPY, bias=-MAGIC)

                    # XBAR transpose -> qT [128(i), KT, 128(t)]
                    qt = qt_pool.tile([P, KT, P], BF16, tag="qt")
                    tq = (nc.scalar if os.environ.get("KERNEL_TQ") == "scalar"
                          else nc.sync)
                    tq.dma_start_transpose(qt[:], qb[:])

                    orow = orow_pool.tile([P, O_CORE], F32, tag="orow")
                    for oc in range(OC):
                        py = psum_y.tile([P, 512], F32, tag="py")
                        for k in range(KT):
                            nc.tensor.matmul(
                                py[:], qt[:, k, :],
                                wtq[oc][k // QK][:, k % QK, :],
                                start=(k == 0), stop=(k == KT - 1),
                            )
                        nc.scalar.activation(
                            orow[:, oc * 512:(oc + 1) * 512], py[:],
                            ACT_COPY, scale=s_t[:],
                        )
                        nc.vector.tensor_tensor(
                            orow[:, oc * 512:(oc + 1) * 512],
                            orow[:, oc * 512:(oc + 1) * 512],
                            bias_bcast[:, oc * 512:(oc + 1) * 512],
                            mybir.AluOpType.add,
                        )
                    nc.sync.dma_start(y_d[tt * P:(tt + 1) * P, :], orow[:])

                # ---- W-prep first: tile deps are inferred from program
                # order, so every wtq write must precede its MM reads.
                # Queue split keeps this from head-of-line blocking the
                # token pipeline: W DMAs ride gpsimd, W XBARs ride sync,
                # token qb/qt ride scalar.
                emit_x_load(2)
                emit_x_load(3)
                for stripe in range(4 * n_str):
                    emit_stripe(stripe)
                for tt in range(TT):
                    emit_x_load(tt + 4)
                    emit_token_tile(tt)

    nc.compile()
    return nc


_NC_CACHE = None


def _get_nc():
    global _NC_CACHE
    if _NC_CACHE is None:
        _NC_CACHE = build_kernel()
    return _NC_CACHE


def kernel(x, W_int, scale_i32, bias, _trace=False, _tmpdir=None):
    nc = _get_nc()
    x2 = np.ascontiguousarray(x, dtype=np.float32).reshape(TOKENS, IN_F)
    W_i8 = np.asarray(W_int).astype(np.int8)          # lossless: [-127,127]
    scale_i32 = np.asarray(scale_i32, dtype=np.int32)
    bias2 = np.asarray(bias, dtype=np.float32).reshape(1, OUT_F)

    in_maps = []
    for c in range(N_CORES):
        tg, og = c // OG, c % OG
        in_maps.append({
            "x": np.ascontiguousarray(x2[tg * T_CORE:(tg + 1) * T_CORE]),
            "w": np.ascontiguousarray(W_i8[og * O_CORE:(og + 1) * O_CORE]),
            "s": np.ascontiguousarray(
                scale_i32[og * O_CORE:(og + 1) * O_CORE]),
            "b": np.ascontiguousarray(bias2[:, og * O_CORE:(og + 1) * O_CORE]),
        })

    res = run_bass_kernel_spmd(
        nc, in_maps, core_ids=list(range(N_CORES)),
        trace=_trace, tmpdir=_tmpdir,
    )
    y = np.empty((TOKENS, OUT_F), dtype=np.float32)
    for c in range(N_CORES):
        tg, og = c // OG, c % OG
        y[tg * T_CORE:(tg + 1) * T_CORE, og * O_CORE:(og + 1) * O_CORE] = \
            res.results[c]["y"]
    out = y.reshape(4, 2048, OUT_F)
    if _trace:
        return out, res
    return out
